# revision 30
# baseline (speedup 1.0000x reference)
"""CAN per-sample 2-layer MLP kernel for Trainium2 (8 NeuronCores, SPMD).

Computation (per sample b):
    x = user_emb[b]                           # (50, 16)
    W0, b0, W1, b1 unpacked from item_emb[b]  # (16,16),(16,),(16,16),(16,)
    y = relu(relu(x @ W0 + b0) @ W1 + b1)     # (50, 16)

Mapping (v2 — block-diagonal 7-sample groups, full 128-wide PE):
  * Pure data parallel over 8 cores (2048 samples each, padded to 2058 =
    294 groups of 7).
  * Per group: x̃stack [119 x 50] stacks 7 samples' x^T with a ones row per
    sample (homogeneous coords: bias folded into the weights); BD0
    [119 x 119] block-diagonal [[W0,0],[b0,1]] blocks; BD1 [119 x 112]
    block-diagonal [[W1],[b1]] blocks. Both lhsT APs read 128 cols (the
    overrun into the neighbor region only feeds unread psum/h̃ rows).
  * L1: matmul(ps1, BD0[128x128], x̃[128x50]) — K=M=128 full-array: the
    measured per-matmul cost collapses to ~50ns (FWL weight load overlapped;
    partial-K/M matmuls stream at 1.2GHz instead and cost 2-4x more). Rows
    119-127 ship as host zeros so the K-padding contracts 0s; 119/120-row
    DMAs measured 4-byte-packet floods or device errors — keep 128 rows.
  * relu1 on ACT -> h̃ (bf16, ones rows self-propagate); L2: matmul(ps2,
    BD1[128x128], h̃[128x50]) — BD1's 16-col AP overrun reads neighbor data
    into psum rows 112-127, which nothing reads. relu2 on DVE -> yt bf16.
  * One input DMA per batch of 21 groups (128 descriptors, 12KB runs,
    SWDGE via gpsimd), 3-batch prefetch; per-batch output DMA on sync.
    The kernel is DMA-bound: ~25MB/core at ~300 B/ns observed.
  * Post-passes: strip covered waits, coalesce per-matmul sem-incs (runs
    complete in pc order), split excess inline waits onto NoOps (walrus
    caps DMACopy/Matmult at 1 inline wait).
"""

from contextlib import ExitStack

import numpy as np

import concourse.bass as bass
import concourse.mybir as mybir
from concourse import tile
from concourse.bass_utils import run_bass_kernel_spmd

# Problem constants (hardcoded per contract)
B, N, D = 16384, 50, 16
NCORES = 8
BC = B // NCORES            # 2048 samples per core
GRP = 6                     # samples per block-diag group (16-row blocks)
KB = GRP * D                # 96 feature rows per group
KG = KB + 1                 # +1 shared bias/ones row (row 96)
KD = 112                    # rows shipped per batch: 97 data + 15 zero pads.
                            # DMA partition counts must be multiples of 16:
                            # 119/120-row DMAs measured 4-byte packet floods
                            # (~2x slower) or device errors; 112/128 are clean.
NG = 342                    # groups per core (2052 sample slots, 4 padded)
GB = 18                     # groups per DMA batch
NB = NG // GB               # 19 batches
SB = 6                      # groups per PSUM sub-batch (one bank: 300 f32)
NSUB = GB // SB             # 3 sub-batches per batch
W0C = KG                    # BD0 cols (96 W-cols + 1 ones-col); AP reads 128
W1C = KB                    # BD1 cols (96); AP reads 128, overrun is benign
CW = N + W0C + W1C          # 243 cols per group slot
ROWW = GB * CW + 42         # 4416: +32 for BD1 AP overrun, padded to x64

F32 = mybir.dt.float32
BF16 = mybir.dt.bfloat16


def _strip_covered_waits(nc):
    """Remove, from DMACopy instructions, semaphore waits already guaranteed
    by an earlier instruction on the same engine queue. Coverage is killed
    for a sem from the point of any non-increment update (barrier resets)."""
    for fn in nc.m.functions:
        for blk in fn.blocks:
            seen = {}
            for ins in blk.instructions:
                si = ins.sync_info
                if si is None:
                    continue
                eng = ins.engine
                strippable = type(ins).__name__ == "InstDMACopy"
                kept = []
                changed = False
                for w in si.on_wait:
                    if (
                        strippable
                        and w.wait_mode == "sem-ge-imm"
                        and w.wait_reg is None
                        and seen.get((eng, w.id), -1) >= w.wait_value
                    ):
                        changed = True
                        continue
                    kept.append(w)
                for w in kept:
                    if w.wait_mode == "sem-ge-imm" and w.wait_reg is None:
                        key = (eng, w.id)
                        if seen.get(key, -1) < w.wait_value:
                            seen[key] = w.wait_value
                for u in si.on_update:
                    if u.update_mode != "sem-add-imm" or (
                        u.update_value is not None and u.update_value < 0
                    ):
                        for key in [k for k in seen if k[1] == u.id]:
                            del seen[key]
                if changed:
                    ins.sync_info = mybir.SyncInfo(
                        on_wait=kept, on_update=si.on_update
                    )


def _coalesce_mm_incs(nc, chunk=7):
    """Fold per-matmul sem-inc completion updates into one add-imm on the
    last matmult of each run (runs = consecutive matmults writing the same
    PSUM tile; matmults complete in pc order so waiters see correct counts,
    just no earlier than the run end — which is where the deps point)."""
    for fn in nc.m.functions:
        for blk in fn.blocks:
            runs, cur, cur_key = [], [], object()
            for ins in blk.instructions:
                if type(ins).__name__ == "InstMatmult":
                    ba = getattr(ins.outs[0], "bass_ap", None)
                    t = getattr(ba, "tensor", None) if ba is not None else None
                    key = t.name if t is not None else None
                    if key != cur_key:
                        if cur:
                            runs.append(cur)
                        cur, cur_key = [], key
                    cur.append(ins)
                else:
                    if cur:
                        runs.append(cur)
                    cur, cur_key = [], object()
            if cur:
                runs.append(cur)
            for run in runs:
                for i in range(0, len(run), chunk):
                    sub = run[i : i + chunk]
                    if len(sub) < 2:
                        continue
                    ok = True
                    for ins in sub:
                        si = ins.sync_info
                        for u in si.on_update if si else []:
                            if (
                                u.update_mode not in ("sem-inc", "sem-add-imm")
                                or u.update_reg is not None
                                or (
                                    u.update_mode == "sem-add-imm"
                                    and (u.update_value or 0) < 0
                                )
                            ):
                                ok = False
                    if not ok:
                        continue
                    total = {}
                    for ins in sub[:-1]:
                        si = ins.sync_info
                        if si is None:
                            continue
                        for u in si.on_update:
                            v = 1 if u.update_mode == "sem-inc" else (u.update_value or 0)
                            total[u.id] = total.get(u.id, 0) + v
                        ins.sync_info = mybir.SyncInfo(on_wait=si.on_wait, on_update=[])
                    last = sub[-1]
                    si = last.sync_info
                    waits = si.on_wait if si else []
                    newups = []
                    for u in si.on_update if si else []:
                        v = 1 if u.update_mode == "sem-inc" else (u.update_value or 0)
                        v += total.pop(u.id, 0)
                        newups.append(
                            mybir.SyncUpdate(
                                update_mode="sem-add-imm",
                                id=u.id,
                                update_value=v,
                                sync_type=u.sync_type,
                            )
                        )
                    for sid, v in total.items():
                        newups.append(
                            mybir.SyncUpdate(
                                update_mode="sem-add-imm",
                                id=sid,
                                update_value=v,
                                sync_type="semaphore",
                            )
                        )
                    last.sync_info = mybir.SyncInfo(on_wait=waits, on_update=newups)


_WS_COUNT = [0]


def _split_excess_waits(nc, cap=1):
    """Move excess inline waits onto NoOps inserted immediately before, on
    the same engine queue - semantically identical (sequencers execute
    waits in order)."""
    for fn in nc.m.functions:
        for blk in fn.blocks:
            insts = blk.instructions
            i = 0
            while i < len(insts):
                ins = insts[i]
                si = ins.sync_info
                if si is None or len(si.on_wait) <= cap:
                    i += 1
                    continue
                waits = list(si.on_wait)
                keep, extra = waits[-cap:], waits[:-cap]
                ins.sync_info = mybir.SyncInfo(on_wait=keep, on_update=si.on_update)
                for w in extra:
                    _WS_COUNT[0] += 1
                    nop = mybir.InstNoOp(name=f"I-ws{_WS_COUNT[0]}", ins=[], outs=[])
                    nop.engine = ins.engine
                    nop.sync_info = mybir.SyncInfo(on_wait=[w], on_update=[])
                    insts.insert(i, nop)
                    i += 1
                i += 1


def build_nc(sim_mode=False):
    """Per-core Bass program.

    DRAM:
      ch [NB, KG, ROWW]  bf16: batch bi, row r<119, group slot gg (col gg*CW):
                         x̃ (50) | BD0 (128) | BD1 (112); +16 pad cols.
      yh [NB, GRP*D, GB*N] bf16: y^T rows (7 samples x 16 feats) per group.
    """
    nc = bass.Bass(
        "TRN2",
        target_bir_lowering=False,
        debug=False,
        detect_race_conditions=False,  # post-pass NoOps confuse its bookkeeping
    )
    ch = nc.dram_tensor("ch", [NB, KD, ROWW], BF16, kind="ExternalInput")
    yh = nc.dram_tensor("yh", [NB, GRP * D, GB * N], BF16, kind="ExternalOutput")

    relu = mybir.ActivationFunctionType.Relu
    sf = SB * N                 # 350 psum cols per sub-batch

    with tile.TileContext(nc) as tc, ExitStack() as ctx:
        cpool = ctx.enter_context(tc.tile_pool(name="cpool", bufs=5))
        hpool = ctx.enter_context(tc.tile_pool(name="hpool", bufs=3))
        ypool = ctx.enter_context(tc.tile_pool(name="ypool", bufs=2))
        pspool = ctx.enter_context(tc.tile_pool(name="ps", bufs=4, space="PSUM"))

        prev_sp = [None]

        def sp_chain(inst):
            if prev_sp[0] is not None:
                from concourse.tile_rust import add_dep_helper

                add_dep_helper(inst.ins, prev_sp[0].ins, sync=False,
                               reason="SP issue order")
            prev_sp[0] = inst
            return inst

        cts = {}

        def emit_in_dma(bi):
            ct = cpool.tile([128, ROWW], BF16, name="ct")
            cts[bi] = ct
            # All 128 rows come from DRAM (rows 119-127 are host zeros: the
            # K-padding must contract 0s, and a zeroing memset on-chip would
            # cost ~6us per buffer). First batch is on the critical path:
            # split column-wise so the earliest sub-batches arrive sooner.
            nsplit = 3 if bi == 0 else 1  # 3 divides GB*CW=4374 exactly
            csz = (GB * CW) // nsplit
            for sp in range(nsplit):
                # 128-partition transfers only: SWDGE descriptor generation
                # measured pathological at 119/120 partitions (4-byte packet
                # floods / device errors); rows 119-127 ship as host zeros.
                nc.gpsimd.dma_start(
                    bass.AP(ct.tensor, sp * csz, [[ROWW, KD], [1, csz]]),
                    bass.AP(ch, bi * KD * ROWW + sp * csz, [[ROWW, KD], [1, csz]]),
                )

        PB = 3
        for pb in range(min(PB, NB)):
            emit_in_dma(pb)

        subs = [(bi, s) for bi in range(NB) for s in range(NSUB)]
        state = {}
        yts = {}

        def emit_l1(ss):
            bi, s = subs[ss]
            if s == 0:
                if bi + PB < NB:
                    emit_in_dma(bi + PB)
                yts[bi] = ypool.tile([128, GB * N], BF16, name="yt")
            ct = cts[bi]
            ps1 = pspool.tile([128, sf], F32, name="ps1")
            if sim_mode:
                nc.vector.memset(ps1[:, :], 0.0)
            for g in range(SB):
                gg = s * SB + g
                nc.tensor.matmul(
                    bass.AP(ps1.tensor, g * N, [[sf, 128], [1, N]]),
                    bass.AP(ct.tensor, gg * CW + N, [[ROWW, KD], [1, 128]]),
                    bass.AP(ct.tensor, gg * CW, [[ROWW, KD], [1, N]]),
                    start=True,
                    stop=True,
                )
            ht = hpool.tile([128, sf], BF16, name="ht")
            nc.scalar.activation(ht[:, :], ps1[:, :], relu)
            state[ss] = (ct, ht)

        def emit_l2(ss):
            bi, s = subs[ss]
            ct, ht = state.pop(ss)
            yt = yts[bi]
            ps2 = pspool.tile([128, sf], F32, name="ps2")
            if sim_mode:
                nc.vector.memset(ps2[:, :], 0.0)
            for g in range(SB):
                gg = s * SB + g
                nc.tensor.matmul(
                    bass.AP(ps2.tensor, g * N, [[sf, 128], [1, N]]),
                    bass.AP(ct.tensor, gg * CW + N + W0C, [[ROWW, KD], [1, 128]]),
                    bass.AP(ht.tensor, g * N, [[sf, KD], [1, N]]),
                    start=True,
                    stop=True,
                )
            nc.vector.tensor_scalar_max(
                bass.AP(yt.tensor, s * sf, [[GB * N, 128], [1, sf]]),
                ps2[:, :],
                0.0,
            )
            if s == NSUB - 1:
                sp_chain(nc.sync.dma_start(
                    bass.AP(yh, bi * GRP * D * GB * N, [[GB * N, GRP * D], [1, GB * N]]),
                    bass.AP(yt.tensor, 0, [[GB * N, GRP * D], [1, GB * N]]),
                ))
                cts.pop(bi)

        SKEW = 1
        for idx in range(len(subs) + SKEW):
            if idx < len(subs):
                emit_l1(idx)
            if idx >= SKEW:
                emit_l2(idx - SKEW)

    _strip_covered_waits(nc)
    _coalesce_mm_incs(nc)
    _split_excess_waits(nc)
    return nc


def pack_inputs(user_emb, item_emb, dt=None):
    """Shard + lay out inputs for the 8 cores (block-diag-7 layout)."""
    if dt is None:
        import ml_dtypes

        dt = ml_dtypes.bfloat16
    x = np.ascontiguousarray(user_emb, dtype=np.float32)
    ie = np.ascontiguousarray(item_emb, dtype=np.float32)

    nslot = NCORES * NG * GRP               # 16464 sample slots
    xp = np.zeros((nslot, N, D), dtype=np.float32)
    wp = np.zeros((nslot, ie.shape[1]), dtype=np.float32)
    # core c's slots are samples c*BC .. c*BC+2047 padded to 2058
    src = np.arange(B).reshape(NCORES, BC)
    slot = (np.arange(NCORES * NG * GRP).reshape(NCORES, NG * GRP))[:, :BC]
    xp[slot.ravel()] = x[src.ravel()]
    wp[slot.ravel()] = ie[src.ravel()]

    g = nslot // GRP                        # total groups
    A = np.zeros((g, KG, CW), dtype=np.float32)
    sidx = np.arange(nslot).reshape(g, GRP)
    # x̃: rows 17s+f = x^T features, row 17s+16 = ones
    xt = xp[sidx].transpose(0, 1, 3, 2)     # (g, GRP, D, N)
    A[:, KB, :N] = 1.0                      # shared ones row (row 96)
    A[:, KB, N + KB] = 1.0                  # BD0 ones col: h̃ row 96 = relu(1)
    for s in range(GRP):
        A[:, D * s : D * s + D, :N] = xt[:, s]
        w = wp[sidx[:, s]]
        W0 = w[:, : D * D].reshape(g, D, D)
        b0 = w[:, D * D : D * D + D]
        off = D * (D + 1)
        W1 = w[:, off : off + D * D].reshape(g, D, D)
        b1 = w[:, off + D * D : off + D * D + D]
        c0 = N + D * s
        A[:, D * s : D * s + D, c0 : c0 + D] = W0
        A[:, KB, c0 : c0 + D] = b0          # bias lives on the shared row
        c1 = N + W0C + D * s
        A[:, D * s : D * s + D, c1 : c1 + D] = W1
        A[:, KB, c1 : c1 + D] = b1

    A = A.reshape(NCORES, NB, GB, KG, CW).transpose(0, 1, 3, 2, 4)
    out = []
    for c in range(NCORES):
        ch = np.zeros((NB, KD, ROWW), dtype=dt)
        ch[:, :KG, : GB * CW] = A[c].reshape(NB, KG, GB * CW).astype(dt)
        out.append({"ch": ch})
    return out


def unpack_output(results):
    """results: per-core {"yh": [NB, 112, GB*N]} -> full (B, N, D) f32."""
    yh = np.stack([np.asarray(r["yh"], dtype=np.float32) for r in results])
    # (c, bi, s*16+e, gb*N+n) -> sample slot c*(NG*GRP) + (bi*GB+gb)*GRP + s
    y = (
        yh.reshape(NCORES, NB, GRP, D, GB, N)
        .transpose(0, 1, 4, 2, 5, 3)        # c, bi, gb, s, n, e
        .reshape(NCORES, NG * GRP, N, D)
    )
    return np.ascontiguousarray(y[:, :BC].reshape(B, N, D))


_NC_CACHE = {}


def _get_nc():
    if "nc" not in _NC_CACHE:
        _NC_CACHE["nc"] = build_nc()
    return _NC_CACHE["nc"]


def kernel(user_emb, item_emb):
    nc = _get_nc()
    in_maps = pack_inputs(user_emb, item_emb)
    res = run_bass_kernel_spmd(nc, in_maps, core_ids=list(range(NCORES)))
    return unpack_output(res.results)


# revision 31
# speedup vs baseline: 1.0948x; 1.0948x over previous
"""CAN per-sample 2-layer MLP kernel for Trainium2 (8 NeuronCores, SPMD).

Computation (per sample b):
    x = user_emb[b]                           # (50, 16)
    W0, b0, W1, b1 unpacked from item_emb[b]  # (16,16),(16,),(16,16),(16,)
    y = relu(relu(x @ W0 + b0) @ W1 + b1)     # (50, 16)

Mapping (v2 — block-diagonal 7-sample groups, full 128-wide PE):
  * Pure data parallel over 8 cores (2048 samples each, padded to 2058 =
    294 groups of 7).
  * Per group: x̃stack [119 x 50] stacks 7 samples' x^T with a ones row per
    sample (homogeneous coords: bias folded into the weights); BD0
    [119 x 119] block-diagonal [[W0,0],[b0,1]] blocks; BD1 [119 x 112]
    block-diagonal [[W1],[b1]] blocks. Both lhsT APs read 128 cols (the
    overrun into the neighbor region only feeds unread psum/h̃ rows).
  * L1: matmul(ps1, BD0[128x128], x̃[128x50]) — K=M=128 full-array: the
    measured per-matmul cost collapses to ~50ns (FWL weight load overlapped;
    partial-K/M matmuls stream at 1.2GHz instead and cost 2-4x more). Rows
    119-127 ship as host zeros so the K-padding contracts 0s; 119/120-row
    DMAs measured 4-byte-packet floods or device errors — keep 128 rows.
  * relu1 on ACT -> h̃ (bf16, ones rows self-propagate); L2: matmul(ps2,
    BD1[128x128], h̃[128x50]) — BD1's 16-col AP overrun reads neighbor data
    into psum rows 112-127, which nothing reads. relu2 on DVE -> yt bf16.
  * One input DMA per batch of 21 groups (128 descriptors, 12KB runs,
    SWDGE via gpsimd), 3-batch prefetch; per-batch output DMA on sync.
    The kernel is DMA-bound: ~25MB/core at ~300 B/ns observed.
  * Post-passes: strip covered waits, coalesce per-matmul sem-incs (runs
    complete in pc order), split excess inline waits onto NoOps (walrus
    caps DMACopy/Matmult at 1 inline wait).
"""

from contextlib import ExitStack

import numpy as np

import concourse.bass as bass
import concourse.mybir as mybir
from concourse import tile
from concourse.bass_utils import run_bass_kernel_spmd

# Problem constants (hardcoded per contract)
B, N, D = 16384, 50, 16
NCORES = 8
BC = B // NCORES            # 2048 samples per core
K = D + 1                   # 17 rows per sample (16 features + ones row)
GRP = 7                     # samples per block-diag group
KG = GRP * K                # 119 data rows per group
NG = 294                    # groups per core (2058 sample slots, 10 padded)
GB = 21                     # groups per DMA batch
NB = NG // GB               # 14 batches
SB = 7                      # groups per PSUM sub-batch (one bank: 350 f32)
NSUB = GB // SB             # 3 sub-batches per batch
W0C = KG                    # BD0 cols (119; lhsT AP reads 128, overrun benign)
W1C = GRP * D               # BD1 cols (112; AP reads 128, overrun is benign)
CW = N + W0C + W1C          # 281 cols per group slot
# 21*281+16(overrun) = 5917, padded to 47*128 so every DRAM row and batch
# stride is 256B-aligned -- misaligned strides measured 2x worse DMA
# descriptor aggregation (4320 packets vs 2272 for the same bytes).
ROWW = 6016

F32 = mybir.dt.float32
BF16 = mybir.dt.bfloat16


def _strip_covered_waits(nc):
    """Remove, from DMACopy instructions, semaphore waits already guaranteed
    by an earlier instruction on the same engine queue. Coverage is killed
    for a sem from the point of any non-increment update (barrier resets)."""
    for fn in nc.m.functions:
        for blk in fn.blocks:
            seen = {}
            for ins in blk.instructions:
                si = ins.sync_info
                if si is None:
                    continue
                eng = ins.engine
                strippable = type(ins).__name__ == "InstDMACopy"
                kept = []
                changed = False
                for w in si.on_wait:
                    if (
                        strippable
                        and w.wait_mode == "sem-ge-imm"
                        and w.wait_reg is None
                        and seen.get((eng, w.id), -1) >= w.wait_value
                    ):
                        changed = True
                        continue
                    kept.append(w)
                for w in kept:
                    if w.wait_mode == "sem-ge-imm" and w.wait_reg is None:
                        key = (eng, w.id)
                        if seen.get(key, -1) < w.wait_value:
                            seen[key] = w.wait_value
                for u in si.on_update:
                    if u.update_mode != "sem-add-imm" or (
                        u.update_value is not None and u.update_value < 0
                    ):
                        for key in [k for k in seen if k[1] == u.id]:
                            del seen[key]
                if changed:
                    ins.sync_info = mybir.SyncInfo(
                        on_wait=kept, on_update=si.on_update
                    )


def _coalesce_mm_incs(nc, chunk=7):
    """Fold per-matmul sem-inc completion updates into one add-imm on the
    last matmult of each run (runs = consecutive matmults writing the same
    PSUM tile; matmults complete in pc order so waiters see correct counts,
    just no earlier than the run end — which is where the deps point)."""
    for fn in nc.m.functions:
        for blk in fn.blocks:
            runs, cur, cur_key = [], [], object()
            for ins in blk.instructions:
                if type(ins).__name__ == "InstMatmult":
                    ba = getattr(ins.outs[0], "bass_ap", None)
                    t = getattr(ba, "tensor", None) if ba is not None else None
                    key = t.name if t is not None else None
                    if key != cur_key:
                        if cur:
                            runs.append(cur)
                        cur, cur_key = [], key
                    cur.append(ins)
                else:
                    if cur:
                        runs.append(cur)
                    cur, cur_key = [], object()
            if cur:
                runs.append(cur)
            for run in runs:
                for i in range(0, len(run), chunk):
                    sub = run[i : i + chunk]
                    if len(sub) < 2:
                        continue
                    ok = True
                    for ins in sub:
                        si = ins.sync_info
                        for u in si.on_update if si else []:
                            if (
                                u.update_mode not in ("sem-inc", "sem-add-imm")
                                or u.update_reg is not None
                                or (
                                    u.update_mode == "sem-add-imm"
                                    and (u.update_value or 0) < 0
                                )
                            ):
                                ok = False
                    if not ok:
                        continue
                    total = {}
                    for ins in sub[:-1]:
                        si = ins.sync_info
                        if si is None:
                            continue
                        for u in si.on_update:
                            v = 1 if u.update_mode == "sem-inc" else (u.update_value or 0)
                            total[u.id] = total.get(u.id, 0) + v
                        ins.sync_info = mybir.SyncInfo(on_wait=si.on_wait, on_update=[])
                    last = sub[-1]
                    si = last.sync_info
                    waits = si.on_wait if si else []
                    newups = []
                    for u in si.on_update if si else []:
                        v = 1 if u.update_mode == "sem-inc" else (u.update_value or 0)
                        v += total.pop(u.id, 0)
                        newups.append(
                            mybir.SyncUpdate(
                                update_mode="sem-add-imm",
                                id=u.id,
                                update_value=v,
                                sync_type=u.sync_type,
                            )
                        )
                    for sid, v in total.items():
                        newups.append(
                            mybir.SyncUpdate(
                                update_mode="sem-add-imm",
                                id=sid,
                                update_value=v,
                                sync_type="semaphore",
                            )
                        )
                    last.sync_info = mybir.SyncInfo(on_wait=waits, on_update=newups)


_WS_COUNT = [0]


def _split_excess_waits(nc, cap=1):
    """Move excess inline waits onto NoOps inserted immediately before, on
    the same engine queue - semantically identical (sequencers execute
    waits in order)."""
    for fn in nc.m.functions:
        for blk in fn.blocks:
            insts = blk.instructions
            i = 0
            while i < len(insts):
                ins = insts[i]
                si = ins.sync_info
                if si is None or len(si.on_wait) <= cap:
                    i += 1
                    continue
                waits = list(si.on_wait)
                keep, extra = waits[-cap:], waits[:-cap]
                ins.sync_info = mybir.SyncInfo(on_wait=keep, on_update=si.on_update)
                for w in extra:
                    _WS_COUNT[0] += 1
                    nop = mybir.InstNoOp(name=f"I-ws{_WS_COUNT[0]}", ins=[], outs=[])
                    nop.engine = ins.engine
                    nop.sync_info = mybir.SyncInfo(on_wait=[w], on_update=[])
                    insts.insert(i, nop)
                    i += 1
                i += 1


def build_nc(sim_mode=False):
    """Per-core Bass program.

    DRAM:
      ch [NB, KG, ROWW]  bf16: batch bi, row r<119, group slot gg (col gg*CW):
                         x̃ (50) | BD0 (128) | BD1 (112); +16 pad cols.
      yh [NB, GRP*D, GB*N] bf16: y^T rows (7 samples x 16 feats) per group.
    """
    nc = bass.Bass(
        "TRN2",
        target_bir_lowering=False,
        debug=False,
        detect_race_conditions=False,  # post-pass NoOps confuse its bookkeeping
    )
    ch = nc.dram_tensor("ch", [NB, 128, ROWW], BF16, kind="ExternalInput")
    yh = nc.dram_tensor("yh", [NB, GRP * D, GB * N], BF16, kind="ExternalOutput")

    relu = mybir.ActivationFunctionType.Relu
    sf = SB * N                 # 350 psum cols per sub-batch

    with tile.TileContext(nc) as tc, ExitStack() as ctx:
        cpool = ctx.enter_context(tc.tile_pool(name="cpool", bufs=5))
        hpool = ctx.enter_context(tc.tile_pool(name="hpool", bufs=3))
        ypool = ctx.enter_context(tc.tile_pool(name="ypool", bufs=2))
        pspool = ctx.enter_context(tc.tile_pool(name="ps", bufs=4, space="PSUM"))

        prev_sp = [None]

        def sp_chain(inst):
            if prev_sp[0] is not None:
                from concourse.tile_rust import add_dep_helper

                add_dep_helper(inst.ins, prev_sp[0].ins, sync=False,
                               reason="SP issue order")
            prev_sp[0] = inst
            return inst

        cts = {}

        def emit_in_dma(bi):
            ct = cpool.tile([128, ROWW], BF16, name="ct")
            cts[bi] = ct
            # All 128 rows come from DRAM (rows 119-127 are host zeros: the
            # K-padding must contract 0s, and a zeroing memset on-chip would
            # cost ~6us per buffer). First batch is on the critical path:
            # split column-wise so the earliest sub-batches arrive sooner.
            nsplit = 3 if bi == 0 else 1  # 3 divides GB*CW=5901 exactly
            csz = (GB * CW) // nsplit
            for sp in range(nsplit):
                # 128-partition transfers only: SWDGE descriptor generation
                # measured pathological at 119/120 partitions (4-byte packet
                # floods / device errors); rows 119-127 ship as host zeros.
                nc.gpsimd.dma_start(
                    bass.AP(ct.tensor, sp * csz, [[ROWW, 128], [1, csz]]),
                    bass.AP(ch, bi * 128 * ROWW + sp * csz, [[ROWW, 128], [1, csz]]),
                )

        PB = 3
        for pb in range(min(PB, NB)):
            emit_in_dma(pb)

        subs = [(bi, s) for bi in range(NB) for s in range(NSUB)]
        state = {}
        yts = {}

        def emit_l1(ss):
            bi, s = subs[ss]
            if s == 0:
                if bi + PB < NB:
                    emit_in_dma(bi + PB)
                yts[bi] = ypool.tile([128, GB * N], BF16, name="yt")
            ct = cts[bi]
            ps1 = pspool.tile([128, sf], F32, name="ps1")
            if sim_mode:
                nc.vector.memset(ps1[:, :], 0.0)
            for g in range(SB):
                gg = s * SB + g
                nc.tensor.matmul(
                    bass.AP(ps1.tensor, g * N, [[sf, 128], [1, N]]),
                    bass.AP(ct.tensor, gg * CW + N, [[ROWW, 128], [1, 128]]),
                    bass.AP(ct.tensor, gg * CW, [[ROWW, 128], [1, N]]),
                    start=True,
                    stop=True,
                )
            ht = hpool.tile([128, sf], BF16, name="ht")
            nc.scalar.activation(ht[:, :], ps1[:, :], relu)
            state[ss] = (ct, ht)

        def emit_l2(ss):
            bi, s = subs[ss]
            ct, ht = state.pop(ss)
            yt = yts[bi]
            ps2 = pspool.tile([128, sf], F32, name="ps2")
            if sim_mode:
                nc.vector.memset(ps2[:, :], 0.0)
            for g in range(SB):
                gg = s * SB + g
                nc.tensor.matmul(
                    bass.AP(ps2.tensor, g * N, [[sf, 128], [1, N]]),
                    bass.AP(ct.tensor, gg * CW + N + W0C, [[ROWW, 128], [1, 128]]),
                    bass.AP(ht.tensor, g * N, [[sf, 128], [1, N]]),
                    start=True,
                    stop=True,
                )
            nc.vector.tensor_scalar_max(
                bass.AP(yt.tensor, s * sf, [[GB * N, 128], [1, sf]]),
                ps2[:, :],
                0.0,
            )
            if s == NSUB - 1:
                sp_chain(nc.sync.dma_start(
                    bass.AP(yh, bi * GRP * D * GB * N, [[GB * N, GRP * D], [1, GB * N]]),
                    bass.AP(yt.tensor, 0, [[GB * N, GRP * D], [1, GB * N]]),
                ))
                cts.pop(bi)

        SKEW = 1
        for idx in range(len(subs) + SKEW):
            if idx < len(subs):
                emit_l1(idx)
            if idx >= SKEW:
                emit_l2(idx - SKEW)

    _strip_covered_waits(nc)
    _coalesce_mm_incs(nc)
    _split_excess_waits(nc)
    return nc


def pack_inputs(user_emb, item_emb, dt=None):
    """Shard + lay out inputs for the 8 cores (block-diag-7 layout)."""
    if dt is None:
        import ml_dtypes

        dt = ml_dtypes.bfloat16
    x = np.ascontiguousarray(user_emb, dtype=np.float32)
    ie = np.ascontiguousarray(item_emb, dtype=np.float32)

    nslot = NCORES * NG * GRP               # 16464 sample slots
    xp = np.zeros((nslot, N, D), dtype=np.float32)
    wp = np.zeros((nslot, ie.shape[1]), dtype=np.float32)
    # core c's slots are samples c*BC .. c*BC+2047 padded to 2058
    src = np.arange(B).reshape(NCORES, BC)
    slot = (np.arange(NCORES * NG * GRP).reshape(NCORES, NG * GRP))[:, :BC]
    xp[slot.ravel()] = x[src.ravel()]
    wp[slot.ravel()] = ie[src.ravel()]

    g = nslot // GRP                        # total groups
    A = np.zeros((g, KG, CW), dtype=np.float32)
    sidx = np.arange(nslot).reshape(g, GRP)
    # x̃: rows 17s+f = x^T features, row 17s+16 = ones
    xt = xp[sidx].transpose(0, 1, 3, 2)     # (g, GRP, D, N)
    for s in range(GRP):
        A[:, 17 * s : 17 * s + D, :N] = xt[:, s]
        A[:, 17 * s + D, :N] = 1.0
        w = wp[sidx[:, s]]
        W0 = w[:, : D * D].reshape(g, D, D)
        b0 = w[:, D * D : D * D + D]
        off = D * (D + 1)
        W1 = w[:, off : off + D * D].reshape(g, D, D)
        b1 = w[:, off + D * D : off + D * D + D]
        c0 = N + 17 * s
        A[:, 17 * s : 17 * s + D, c0 : c0 + D] = W0
        A[:, 17 * s + D, c0 : c0 + D] = b0
        A[:, 17 * s + D, c0 + D] = 1.0      # ones col propagates the ones row
        c1 = N + W0C + D * s
        A[:, 17 * s : 17 * s + D, c1 : c1 + D] = W1
        A[:, 17 * s + D, c1 : c1 + D] = b1

    A = A.reshape(NCORES, NB, GB, KG, CW).transpose(0, 1, 3, 2, 4)
    out = []
    for c in range(NCORES):
        ch = np.zeros((NB, 128, ROWW), dtype=dt)
        ch[:, :KG, : GB * CW] = A[c].reshape(NB, KG, GB * CW).astype(dt)
        out.append({"ch": ch})
    return out


def unpack_output(results):
    """results: per-core {"yh": [NB, 112, GB*N]} -> full (B, N, D) f32."""
    yh = np.stack([np.asarray(r["yh"], dtype=np.float32) for r in results])
    # (c, bi, s*16+e, gb*N+n) -> sample slot c*(NG*GRP) + (bi*GB+gb)*GRP + s
    y = (
        yh.reshape(NCORES, NB, GRP, D, GB, N)
        .transpose(0, 1, 4, 2, 5, 3)        # c, bi, gb, s, n, e
        .reshape(NCORES, NG * GRP, N, D)
    )
    return np.ascontiguousarray(y[:, :BC].reshape(B, N, D))


_NC_CACHE = {}


def _get_nc():
    if "nc" not in _NC_CACHE:
        _NC_CACHE["nc"] = build_nc()
    return _NC_CACHE["nc"]


def kernel(user_emb, item_emb):
    nc = _get_nc()
    in_maps = pack_inputs(user_emb, item_emb)
    res = run_bass_kernel_spmd(nc, in_maps, core_ids=list(range(NCORES)))
    return unpack_output(res.results)


# revision 32
# speedup vs baseline: 1.1363x; 1.0379x over previous
"""CAN per-sample 2-layer MLP kernel for Trainium2 (8 NeuronCores, SPMD).

Computation (per sample b):
    x = user_emb[b]                           # (50, 16)
    W0, b0, W1, b1 unpacked from item_emb[b]  # (16,16),(16,),(16,16),(16,)
    y = relu(relu(x @ W0 + b0) @ W1 + b1)     # (50, 16)

Mapping (v2 — block-diagonal 7-sample groups, full 128-wide PE):
  * Pure data parallel over 8 cores (2048 samples each, padded to 2058 =
    294 groups of 7).
  * Per group: x̃stack [119 x 50] stacks 7 samples' x^T with a ones row per
    sample (homogeneous coords: bias folded into the weights); BD0
    [119 x 119] block-diagonal [[W0,0],[b0,1]] blocks; BD1 [119 x 112]
    block-diagonal [[W1],[b1]] blocks. Both lhsT APs read 128 cols (the
    overrun into the neighbor region only feeds unread psum/h̃ rows).
  * L1: matmul(ps1, BD0[128x128], x̃[128x50]) — K=M=128 full-array: the
    measured per-matmul cost collapses to ~50ns (FWL weight load overlapped;
    partial-K/M matmuls stream at 1.2GHz instead and cost 2-4x more). Rows
    119-127 ship as host zeros so the K-padding contracts 0s; 119/120-row
    DMAs measured 4-byte-packet floods or device errors — keep 128 rows.
  * relu1 on ACT -> h̃ (bf16, ones rows self-propagate); L2: matmul(ps2,
    BD1[128x128], h̃[128x50]) — BD1's 16-col AP overrun reads neighbor data
    into psum rows 112-127, which nothing reads. relu2 on DVE -> yt bf16.
  * One input DMA per batch of 21 groups (128 descriptors, 12KB runs,
    SWDGE via gpsimd), 3-batch prefetch; per-batch output DMA on sync.
    The kernel is DMA-bound: ~25MB/core at ~300 B/ns observed.
  * Post-passes: strip covered waits, coalesce per-matmul sem-incs (runs
    complete in pc order), split excess inline waits onto NoOps (walrus
    caps DMACopy/Matmult at 1 inline wait).
"""

from contextlib import ExitStack

import numpy as np

import concourse.bass as bass
import concourse.mybir as mybir
from concourse import tile
from concourse.bass_utils import run_bass_kernel_spmd

# Problem constants (hardcoded per contract)
B, N, D = 16384, 50, 16
NCORES = 8
BC = B // NCORES            # 2048 samples per core
K = D + 1                   # 17 rows per sample (16 features + ones row)
GRP = 7                     # samples per block-diag group
KG = GRP * K                # 119 data rows per group
NG = 294                    # groups per core (2058 sample slots, 10 padded)
GB = 21                     # groups per DMA batch
NB = NG // GB               # 14 batches
SB = 7                      # groups per PSUM sub-batch (one bank: 350 f32)
NSUB = GB // SB             # 3 sub-batches per batch
W0C = KG                    # BD0 cols (119; lhsT AP reads 128, overrun benign)
W1C = GRP * D               # BD1 cols (112; AP reads 128, overrun is benign)
CW = N + W0C + W1C          # 281 cols per group slot
# 21*281+16(overrun) = 5917, padded to 47*128 so every DRAM row and batch
# stride is 256B-aligned -- misaligned strides measured 2x worse DMA
# descriptor aggregation (4320 packets vs 2272 for the same bytes).
ROWW = 6016

F32 = mybir.dt.float32
BF16 = mybir.dt.bfloat16


def _strip_covered_waits(nc):
    """Remove, from DMACopy instructions, semaphore waits already guaranteed
    by an earlier instruction on the same engine queue. Coverage is killed
    for a sem from the point of any non-increment update (barrier resets)."""
    for fn in nc.m.functions:
        for blk in fn.blocks:
            seen = {}
            for ins in blk.instructions:
                si = ins.sync_info
                if si is None:
                    continue
                eng = ins.engine
                strippable = type(ins).__name__ == "InstDMACopy"
                kept = []
                changed = False
                for w in si.on_wait:
                    if (
                        strippable
                        and w.wait_mode == "sem-ge-imm"
                        and w.wait_reg is None
                        and seen.get((eng, w.id), -1) >= w.wait_value
                    ):
                        changed = True
                        continue
                    kept.append(w)
                for w in kept:
                    if w.wait_mode == "sem-ge-imm" and w.wait_reg is None:
                        key = (eng, w.id)
                        if seen.get(key, -1) < w.wait_value:
                            seen[key] = w.wait_value
                for u in si.on_update:
                    if u.update_mode != "sem-add-imm" or (
                        u.update_value is not None and u.update_value < 0
                    ):
                        for key in [k for k in seen if k[1] == u.id]:
                            del seen[key]
                if changed:
                    ins.sync_info = mybir.SyncInfo(
                        on_wait=kept, on_update=si.on_update
                    )


def _coalesce_mm_incs(nc, chunk=7):
    """Fold per-matmul sem-inc completion updates into one add-imm on the
    last matmult of each run (runs = consecutive matmults writing the same
    PSUM tile; matmults complete in pc order so waiters see correct counts,
    just no earlier than the run end — which is where the deps point)."""
    for fn in nc.m.functions:
        for blk in fn.blocks:
            runs, cur, cur_key = [], [], object()
            for ins in blk.instructions:
                if type(ins).__name__ == "InstMatmult":
                    ba = getattr(ins.outs[0], "bass_ap", None)
                    t = getattr(ba, "tensor", None) if ba is not None else None
                    key = t.name if t is not None else None
                    if key != cur_key:
                        if cur:
                            runs.append(cur)
                        cur, cur_key = [], key
                    cur.append(ins)
                else:
                    if cur:
                        runs.append(cur)
                    cur, cur_key = [], object()
            if cur:
                runs.append(cur)
            for run in runs:
                for i in range(0, len(run), chunk):
                    sub = run[i : i + chunk]
                    if len(sub) < 2:
                        continue
                    ok = True
                    for ins in sub:
                        si = ins.sync_info
                        for u in si.on_update if si else []:
                            if (
                                u.update_mode not in ("sem-inc", "sem-add-imm")
                                or u.update_reg is not None
                                or (
                                    u.update_mode == "sem-add-imm"
                                    and (u.update_value or 0) < 0
                                )
                            ):
                                ok = False
                    if not ok:
                        continue
                    total = {}
                    for ins in sub[:-1]:
                        si = ins.sync_info
                        if si is None:
                            continue
                        for u in si.on_update:
                            v = 1 if u.update_mode == "sem-inc" else (u.update_value or 0)
                            total[u.id] = total.get(u.id, 0) + v
                        ins.sync_info = mybir.SyncInfo(on_wait=si.on_wait, on_update=[])
                    last = sub[-1]
                    si = last.sync_info
                    waits = si.on_wait if si else []
                    newups = []
                    for u in si.on_update if si else []:
                        v = 1 if u.update_mode == "sem-inc" else (u.update_value or 0)
                        v += total.pop(u.id, 0)
                        newups.append(
                            mybir.SyncUpdate(
                                update_mode="sem-add-imm",
                                id=u.id,
                                update_value=v,
                                sync_type=u.sync_type,
                            )
                        )
                    for sid, v in total.items():
                        newups.append(
                            mybir.SyncUpdate(
                                update_mode="sem-add-imm",
                                id=sid,
                                update_value=v,
                                sync_type="semaphore",
                            )
                        )
                    last.sync_info = mybir.SyncInfo(on_wait=waits, on_update=newups)


_WS_COUNT = [0]


def _split_excess_waits(nc, cap=1):
    """Move excess inline waits onto NoOps inserted immediately before, on
    the same engine queue - semantically identical (sequencers execute
    waits in order)."""
    for fn in nc.m.functions:
        for blk in fn.blocks:
            insts = blk.instructions
            i = 0
            while i < len(insts):
                ins = insts[i]
                si = ins.sync_info
                if si is None or len(si.on_wait) <= cap:
                    i += 1
                    continue
                waits = list(si.on_wait)
                keep, extra = waits[-cap:], waits[:-cap]
                ins.sync_info = mybir.SyncInfo(on_wait=keep, on_update=si.on_update)
                for w in extra:
                    _WS_COUNT[0] += 1
                    nop = mybir.InstNoOp(name=f"I-ws{_WS_COUNT[0]}", ins=[], outs=[])
                    nop.engine = ins.engine
                    nop.sync_info = mybir.SyncInfo(on_wait=[w], on_update=[])
                    insts.insert(i, nop)
                    i += 1
                i += 1


def build_nc(sim_mode=False):
    """Per-core Bass program.

    DRAM:
      ch [NB, KG, ROWW]  bf16: batch bi, row r<119, group slot gg (col gg*CW):
                         x̃ (50) | BD0 (128) | BD1 (112); +16 pad cols.
      yh [NB, GRP*D, GB*N] bf16: y^T rows (7 samples x 16 feats) per group.
    """
    nc = bass.Bass(
        "TRN2",
        target_bir_lowering=False,
        debug=False,
        detect_race_conditions=False,  # post-pass NoOps confuse its bookkeeping
    )
    ch = nc.dram_tensor("ch", [NB, 128, ROWW], BF16, kind="ExternalInput")
    yh = nc.dram_tensor("yh", [NB, GRP * D, GB * N], BF16, kind="ExternalOutput")

    relu = mybir.ActivationFunctionType.Relu
    sf = SB * N                 # 350 psum cols per sub-batch

    with tile.TileContext(nc) as tc, ExitStack() as ctx:
        cpool = ctx.enter_context(tc.tile_pool(name="cpool", bufs=5))
        hpool = ctx.enter_context(tc.tile_pool(name="hpool", bufs=3))
        ypool = ctx.enter_context(tc.tile_pool(name="ypool", bufs=2))
        pspool = ctx.enter_context(tc.tile_pool(name="ps", bufs=4, space="PSUM"))

        prev_sp = [None]

        def sp_chain(inst):
            if prev_sp[0] is not None:
                from concourse.tile_rust import add_dep_helper

                add_dep_helper(inst.ins, prev_sp[0].ins, sync=False,
                               reason="SP issue order")
            prev_sp[0] = inst
            return inst

        cts = {}

        def emit_in_dma(bi):
            ct = cpool.tile([128, ROWW], BF16, name="ct")
            cts[bi] = ct
            # All 128 rows come from DRAM (rows 119-127 are host zeros: the
            # K-padding must contract 0s, and a zeroing memset on-chip would
            # cost ~6us per buffer). First batch is on the critical path:
            # split column-wise so the earliest sub-batches arrive sooner.
            nsplit = 3 if bi == 0 else 1  # 3 divides GB*CW=5901 exactly
            csz = (GB * CW) // nsplit
            for sp in range(nsplit):
                # 128-partition transfers only: SWDGE descriptor generation
                # measured pathological at 119/120 partitions (4-byte packet
                # floods / device errors); rows 119-127 ship as host zeros.
                nc.gpsimd.dma_start(
                    bass.AP(ct.tensor, sp * csz, [[ROWW, 128], [1, csz]]),
                    bass.AP(ch, bi * 128 * ROWW + sp * csz, [[ROWW, 128], [1, csz]]),
                )

        PB = 3
        for pb in range(min(PB, NB)):
            emit_in_dma(pb)

        subs = [(bi, s) for bi in range(NB) for s in range(NSUB)]
        state = {}
        yts = {}

        def emit_l1(ss):
            bi, s = subs[ss]
            if s == 0:
                if bi + PB < NB:
                    emit_in_dma(bi + PB)
                yts[bi] = ypool.tile([128, GB * N], BF16, name="yt")
            ct = cts[bi]
            ps1 = pspool.tile([128, sf], F32, name="ps1")
            if sim_mode:
                nc.vector.memset(ps1[:, :], 0.0)
            for g in range(SB):
                gg = s * SB + g
                nc.tensor.matmul(
                    bass.AP(ps1.tensor, g * N, [[sf, 128], [1, N]]),
                    bass.AP(ct.tensor, gg * CW + N, [[ROWW, 128], [1, 128]]),
                    bass.AP(ct.tensor, gg * CW, [[ROWW, 128], [1, N]]),
                    start=True,
                    stop=True,
                )
            ht = hpool.tile([128, sf], BF16, name="ht")
            nc.scalar.activation(ht[:, :], ps1[:, :], relu)
            state[ss] = (ct, ht)

        def emit_l2(ss):
            bi, s = subs[ss]
            ct, ht = state.pop(ss)
            yt = yts[bi]
            ps2 = pspool.tile([128, sf], F32, name="ps2")
            if sim_mode:
                nc.vector.memset(ps2[:, :], 0.0)
            for g in range(SB):
                gg = s * SB + g
                nc.tensor.matmul(
                    bass.AP(ps2.tensor, g * N, [[sf, 128], [1, N]]),
                    bass.AP(ct.tensor, gg * CW + N + W0C, [[ROWW, 128], [1, 128]]),
                    bass.AP(ht.tensor, g * N, [[sf, 128], [1, N]]),
                    start=True,
                    stop=True,
                )
            nc.vector.tensor_scalar_max(
                bass.AP(yt.tensor, s * sf, [[GB * N, 128], [1, sf]]),
                ps2[:, :],
                0.0,
            )
            if s == NSUB - 1:
                sp_chain(nc.sync.dma_start(
                    bass.AP(yh, bi * GRP * D * GB * N, [[GB * N, GRP * D], [1, GB * N]]),
                    bass.AP(yt.tensor, 0, [[GB * N, GRP * D], [1, GB * N]]),
                ))
                cts.pop(bi)

        SKEW = 2
        for idx in range(len(subs) + SKEW):
            if idx < len(subs):
                emit_l1(idx)
            if idx >= SKEW:
                emit_l2(idx - SKEW)

    _strip_covered_waits(nc)
    _coalesce_mm_incs(nc)
    _split_excess_waits(nc)
    return nc


def pack_inputs(user_emb, item_emb, dt=None):
    """Shard + lay out inputs for the 8 cores (block-diag-7 layout)."""
    if dt is None:
        import ml_dtypes

        dt = ml_dtypes.bfloat16
    x = np.ascontiguousarray(user_emb, dtype=np.float32)
    ie = np.ascontiguousarray(item_emb, dtype=np.float32)

    nslot = NCORES * NG * GRP               # 16464 sample slots
    xp = np.zeros((nslot, N, D), dtype=np.float32)
    wp = np.zeros((nslot, ie.shape[1]), dtype=np.float32)
    # core c's slots are samples c*BC .. c*BC+2047 padded to 2058
    src = np.arange(B).reshape(NCORES, BC)
    slot = (np.arange(NCORES * NG * GRP).reshape(NCORES, NG * GRP))[:, :BC]
    xp[slot.ravel()] = x[src.ravel()]
    wp[slot.ravel()] = ie[src.ravel()]

    g = nslot // GRP                        # total groups
    A = np.zeros((g, KG, CW), dtype=np.float32)
    sidx = np.arange(nslot).reshape(g, GRP)
    # x̃: rows 17s+f = x^T features, row 17s+16 = ones
    xt = xp[sidx].transpose(0, 1, 3, 2)     # (g, GRP, D, N)
    for s in range(GRP):
        A[:, 17 * s : 17 * s + D, :N] = xt[:, s]
        A[:, 17 * s + D, :N] = 1.0
        w = wp[sidx[:, s]]
        W0 = w[:, : D * D].reshape(g, D, D)
        b0 = w[:, D * D : D * D + D]
        off = D * (D + 1)
        W1 = w[:, off : off + D * D].reshape(g, D, D)
        b1 = w[:, off + D * D : off + D * D + D]
        c0 = N + 17 * s
        A[:, 17 * s : 17 * s + D, c0 : c0 + D] = W0
        A[:, 17 * s + D, c0 : c0 + D] = b0
        A[:, 17 * s + D, c0 + D] = 1.0      # ones col propagates the ones row
        c1 = N + W0C + D * s
        A[:, 17 * s : 17 * s + D, c1 : c1 + D] = W1
        A[:, 17 * s + D, c1 : c1 + D] = b1

    A = A.reshape(NCORES, NB, GB, KG, CW).transpose(0, 1, 3, 2, 4)
    out = []
    for c in range(NCORES):
        ch = np.zeros((NB, 128, ROWW), dtype=dt)
        ch[:, :KG, : GB * CW] = A[c].reshape(NB, KG, GB * CW).astype(dt)
        out.append({"ch": ch})
    return out


def unpack_output(results):
    """results: per-core {"yh": [NB, 112, GB*N]} -> full (B, N, D) f32."""
    yh = np.stack([np.asarray(r["yh"], dtype=np.float32) for r in results])
    # (c, bi, s*16+e, gb*N+n) -> sample slot c*(NG*GRP) + (bi*GB+gb)*GRP + s
    y = (
        yh.reshape(NCORES, NB, GRP, D, GB, N)
        .transpose(0, 1, 4, 2, 5, 3)        # c, bi, gb, s, n, e
        .reshape(NCORES, NG * GRP, N, D)
    )
    return np.ascontiguousarray(y[:, :BC].reshape(B, N, D))


_NC_CACHE = {}


def _get_nc():
    if "nc" not in _NC_CACHE:
        _NC_CACHE["nc"] = build_nc()
    return _NC_CACHE["nc"]


def kernel(user_emb, item_emb):
    nc = _get_nc()
    in_maps = pack_inputs(user_emb, item_emb)
    res = run_bass_kernel_spmd(nc, in_maps, core_ids=list(range(NCORES)))
    return unpack_output(res.results)
